# revision 1
# baseline (speedup 1.0000x reference)
"""Causal self-attention (B=1, T=4096, D=1024, H=16) on 8 TRN2 NeuronCores.

Sharding: tensor-parallel over heads — 2 heads per core. Each core computes
Q^T/K^T/V for its 2 heads from the full x, runs causal flash-style attention
fully on-chip, applies its slice of the output projection, and writes a
partial [D, T] (transposed) output. The host sums the 8 partials (the
all-reduce of the out projection) and transposes back.

Layout trick: everything is computed in "transposed" space so no on-device
transposes of activations are needed:
  qT/kT: [128, T] with partitions = (head, head_dim)  (via lhsT = w chunk)
  S^T tile: [128 kv, 512 q] = kT_slice.T-matmul  (kv on partitions)
  p^T = exp(S^T/8) (ACT), causal mask by elementwise multiply
  y^T: [65, 512] accumulated in PSUM via lhsT = [v | 1] (ones col => rowsum)
  outT: [128 d, 512 t] via lhsT = w_out chunk, rhs = y^T stacked
V is needed in natural [t, dh] layout (contraction over kv partitions), so it
is computed as v^T like q/k and transposed on the PE (only 32 transposes).

The emission interleaves projection t-slice ts with attention q-tile i=ts
(whose kv window only needs slices <= ts) so all engines ramp together.

Matmuls use float32r (TF32-like, full-rate for moving dim >= 256).
"""

import numpy as np

T = 4096
D = 1024
H = 16
DH = 64
NCORES = 8
HPC = H // NCORES          # heads per core = 2
CD = HPC * DH              # per-core hidden slice = 128
QT = 512                   # query tile (free dim of S^T matmuls)
KT = 128                   # kv tile (partition dim of S^T)
NQ = T // QT               # 8 big q tiles
TS = 512                   # phase-1 t-slice
NTS = T // TS              # 8 slices
NKC = D // 128             # 8 contraction chunks of d_model

_CACHE = {}


def _build():
    import concourse.bass as bass
    import concourse.tile as tile
    from concourse import bacc, mybir
    from concourse.masks import make_identity

    F32 = mybir.dt.float32
    F32R = mybir.dt.float32r
    AF = mybir.ActivationFunctionType

    nc = bacc.Bacc("TRN2", target_bir_lowering=False, debug=False,
                   num_devices=NCORES)

    xT_d = nc.dram_tensor("xt", [D, T], F32R, kind="ExternalInput").ap()
    wq_d = nc.dram_tensor("wq", [D, CD], F32R, kind="ExternalInput").ap()
    wk_d = nc.dram_tensor("wk", [D, CD], F32R, kind="ExternalInput").ap()
    wv_d = nc.dram_tensor("wv", [D, CD], F32R, kind="ExternalInput").ap()
    bq_d = nc.dram_tensor("bqkv", [3, CD], F32, kind="ExternalInput").ap()
    wo_d = nc.dram_tensor("wo", [CD, D], F32R, kind="ExternalInput").ap()
    outT_d = nc.dram_tensor("outt", [D, T], F32R, kind="ExternalOutput").ap()

    with (
        tile.TileContext(nc) as tc,
        tc.tile_pool(name="persist", bufs=1) as persist,
        tc.tile_pool(name="xt", bufs=2) as xtp,
        tc.tile_pool(name="vtq", bufs=2) as vtqp,
        tc.tile_pool(name="pt", bufs=6) as ptp,
        tc.tile_pool(name="rs", bufs=2) as rsp,
        tc.tile_pool(name="rsb", bufs=2) as rsbp,
        tc.tile_pool(name="ot", bufs=3) as otp,
        tc.tile_pool(name="ps_big", bufs=3, space="PSUM") as psb,
        tc.tile_pool(name="ps_y", bufs=2, space="PSUM") as psy,
    ):
        # earliest critical path: slice-0 x DMA before all constant DMAs
        _pre0 = {}
        _xt0 = xtp.tile([128, NKC, TS], F32R, tag="xt", name="xt0")
        for _k in range(NKC):
            nc.sync.dma_start(out=_xt0[:, _k, :],
                              in_=xT_d[_k * 128:(_k + 1) * 128, 0:TS])
        _pre0["xt"] = _xt0

        # ---------------- constants & persistent tiles ----------------
        ones64 = persist.tile([1, 64], F32R)
        ones64_f = persist.tile([1, 64], F32)
        nc.vector.memset(ones64_f, 1.0)
        nc.vector.tensor_copy(out=ones64, in_=ones64_f)
        # shifted causal masks: keep (kv_p <= q_f - 128*jr)
        masks = persist.tile([128, 4, QT], F32)
        nc.vector.memset(masks, 1.0)
        for jr in range(4):
            nc.gpsimd.affine_select(
                out=masks[:, jr, :], in_=masks[:, jr, :],
                compare_op=mybir.AluOpType.is_ge, fill=0.0,
                base=-128 * jr, pattern=[[1, QT]], channel_multiplier=-1,
            )
        wo_sb = persist.tile([128, NKC, 128], F32R)
        nc.sync.dma_start(out=wo_sb, in_=wo_d.rearrange("p (a m) -> p a m", a=NKC))
        ident = persist.tile([128, 128], F32)
        make_identity(nc, ident)

        # projection weights: [128, chunk, CD] (+ bias row separately)
        wq_sb = persist.tile([128, NKC, CD], F32R)
        wk_sb = persist.tile([128, NKC, CD], F32R)
        wv_sb = persist.tile([128, NKC, CD], F32R)
        nc.sync.dma_start(out=wq_sb, in_=wq_d.rearrange("(a p) m -> p a m", p=128))
        nc.sync.dma_start(out=wk_sb, in_=wk_d.rearrange("(a p) m -> p a m", p=128))
        nc.sync.dma_start(out=wv_sb, in_=wv_d.rearrange("(a p) m -> p a m", p=128))
        bq_sb = persist.tile([128, 3], F32)
        nc.sync.dma_start(out=bq_sb, in_=bq_d.rearrange("a p -> p a"))

        # persistent activations
        qTs = persist.tile([128, T], F32R)   # rows h*64+dh
        kTs = persist.tile([128, T], F32R)
        yTs = persist.tile([128, T], F32R)
        # v natural, both heads, with a ones column per head:
        # free layout [32 kv-tiles, 130]: cols 0:64 v_h0, 64 ones, 65:129 v_h1, 129 ones
        v_sb = persist.tile([128, T // KT, 130], F32R)
        vones_f = persist.tile([128, T // KT, 1], F32)
        nc.vector.memset(vones_f, 1.0)
        nc.vector.tensor_copy(out=v_sb[:, :, 64:65], in_=vones_f)
        nc.vector.tensor_copy(out=v_sb[:, :, 129:130], in_=vones_f)

        def xt_dma(ts):
            sl = slice(ts * TS, (ts + 1) * TS)
            xt_e = xtp.tile([128, NKC, TS], F32R, tag="xt", name=f"xt{ts}")
            for k in range(NKC):
                nc.sync.dma_start(
                    out=xt_e[:, k, :],
                    in_=xT_d[k * 128:(k + 1) * 128, sl])
            return xt_e

        def proj_chunks(ts, pre=None):
            """Emit-later closures for projection slice ts: DMA chunk first,
            then q/k/v matmul groups, then v transpose pairs."""
            sl = slice(ts * TS, (ts + 1) * TS)
            state = {"xt": pre}

            def dma_chunk():
                state["xt"] = xt_dma(ts)

            def mm_group(which, bidx, w_sb, dest):
                def emit():
                    xt_e = state["xt"]
                    ps = psb.tile([128, TS], F32, tag="big",
                                  name=f"p{which}{ts}")
                    for k in range(NKC):
                        nc.tensor.matmul(ps, w_sb[:, k, :], xt_e[:, k, :],
                                         start=(k == 0), stop=(k == NKC - 1))
                    if dest is not None:
                        nc.vector.tensor_scalar_add(
                            out=dest[:, sl], in0=ps,
                            scalar1=bq_sb[:, bidx:bidx + 1])
                    else:
                        vt_q = vtqp.tile([128, TS], F32, tag="vtq",
                                         name=f"vtq{ts}")
                        nc.vector.tensor_scalar_add(
                            out=vt_q, in0=ps,
                            scalar1=bq_sb[:, bidx:bidx + 1])
                        state["vtq"] = vt_q
                return emit

            def tr_pair(pp):
                def emit():
                    vt_q = state["vtq"]
                    trp = psb.tile([128, 2, 128], F32, tag="big",
                                   name=f"tr{ts}p{pp}")
                    for jj2 in range(2):
                        jj = 2 * pp + jj2
                        j = ts * (TS // KT) + jj
                        nc.tensor.transpose(
                            trp[:, jj2, :],
                            vt_q[:, jj * 128:(jj + 1) * 128], ident)
                        nc.vector.tensor_copy(out=v_sb[:, j, 0:64],
                                              in_=trp[:, jj2, 0:64])
                        nc.vector.tensor_copy(out=v_sb[:, j, 65:129],
                                              in_=trp[:, jj2, 64:128])
                return emit

            head = [] if pre is not None else [dma_chunk]
            return head + [
                    mm_group("q", 0, wq_sb, qTs),
                    mm_group("k", 1, wk_sb, kTs),
                    mm_group("v", 2, wv_sb, None),
                    tr_pair(0), tr_pair(1)]

        def attention(i, filler=()):
            filler = list(filler)
            qsl = slice(i * QT, (i + 1) * QT)
            nj = (i + 1) * (QT // KT)           # kv tiles for this q tile
            ngroups = nj // 2
            stride = max(1, ngroups // max(1, len(filler)))
            ys = [psy.tile([65, QT], F32, tag="y", name=f"y{i}h{h}")
                  for h in range(HPC)]
            for g in range(ngroups):
                sgs = [psb.tile([128, 2, QT], F32, tag="big",
                                name=f"sg{i}g{g}h{h}") for h in range(HPC)]
                for jj in range(2):
                    j = 2 * g + jj
                    for h in range(HPC):
                        hs = slice(h * DH, (h + 1) * DH)
                        nc.tensor.matmul(
                            sgs[h][:, jj, :],
                            kTs[hs, j * KT:(j + 1) * KT],
                            qTs[hs, qsl], start=True, stop=True)
                pts = []
                for h in range(HPC):
                    pt = ptp.tile([128, 2, QT], F32R, tag="pt",
                                  name=f"pt{i}g{g}h{h}")
                    nc.scalar.activation(out=pt, in_=sgs[h], func=AF.Exp,
                                         scale=0.125)
                    pts.append(pt)
                for h in range(HPC):
                    pt = pts[h]
                    for jj in range(2):
                        j = 2 * g + jj
                        jr = j - (QT // KT) * i
                        if jr >= 0:   # diagonal tile: apply causal mask
                            nc.vector.tensor_mul(
                                out=pt[:, jj, :], in0=pt[:, jj, :],
                                in1=masks[:, jr, :])
                        f0 = max(0, 128 * jr)
                        nc.tensor.matmul(
                            ys[h][:, f0:], v_sb[:, j, 65 * h:65 * h + 65],
                            pt[:, jj, f0:], start=(j == 0),
                            stop=(j == nj - 1))
                if filler and (g % stride == stride - 1 or g == ngroups - 1):
                    filler.pop(0)()
            while filler:
                filler.pop(0)()
            # normalize: y[0:64] / y[64] -> yTs
            rb_ps = psb.tile([128, 2, QT], F32, tag="big", name=f"rb{i}")
            for h in range(HPC):
                rs = rsp.tile([1, QT], F32R, tag="rs", name=f"rs{i}h{h}")
                with nc.allow_low_precision(reason="tf32-grade kernel"):
                    nc.vector.reciprocal(out=rs, in_=ys[h][64:65, :])
                nc.tensor.matmul(rb_ps[0:64, h, :], ones64, rs,
                                 start=True, stop=True)
                rsb = rsbp.tile([64, QT], F32, tag="rsb", name=f"rsb{i}h{h}")
                nc.vector.tensor_copy(out=rsb, in_=rb_ps[0:64, h, :])
                nc.vector.tensor_mul(
                    out=yTs[h * DH:(h + 1) * DH, qsl],
                    in0=ys[h][0:64, :], in1=rsb)
            # out projection chunks for this t slice (emitted later as filler)
            outT_r = outT_d.rearrange("(a p) t -> p a t", p=128)

            def po_chunk(dp):
                def emit():
                    po = psb.tile([128, 2, QT], F32, tag="big",
                                  name=f"po{i}d{dp}")
                    for d2 in range(2):
                        d = 2 * dp + d2
                        nc.tensor.matmul(po[:, d2, :], wo_sb[:, d, :],
                                         yTs[:, qsl], start=True, stop=True)
                    ot = otp.tile([128, 2, QT], F32R, tag="ot",
                                  name=f"ot{i}d{dp}")
                    nc.vector.tensor_copy(out=ot, in_=po)
                    nc.sync.dma_start(
                        out=outT_r[:, 2 * dp:2 * dp + 2, qsl], in_=ot)
                return emit

            return [po_chunk(dp) for dp in range(NKC // 2)]

        # proj slice 0 up front; proj slice i+1 and outproj(i-1) are
        # interleaved into attention(i) as filler chunks
        for ch in proj_chunks(0, pre=_pre0["xt"]):
            ch()
        carry = []
        for i in range(NQ):
            nxt = proj_chunks(i + 1) if i + 1 < NTS else []
            # DMA first, then alternate outproj(i-1) and proj(i+1) chunks
            filler = nxt[:1]
            rest = nxt[1:]
            while carry or rest:
                if carry:
                    filler.append(carry.pop(0))
                if rest:
                    filler.append(rest.pop(0))
            carry = attention(i, filler)
        for ch in carry:
            ch()

    nc.compile()
    return nc


def _prep_inputs(x, w_qkv, b_qkv, w_out, b_out):
    x = np.ascontiguousarray(np.asarray(x, dtype=np.float32).reshape(T, D))
    w_qkv = np.asarray(w_qkv, dtype=np.float32)
    b_qkv = np.asarray(b_qkv, dtype=np.float32)
    w_out = np.asarray(w_out, dtype=np.float32)
    b_out = np.asarray(b_out, dtype=np.float32)

    xT = np.ascontiguousarray(x.T)

    in_maps = []
    for c in range(NCORES):
        h0 = HPC * c
        cols = np.arange(h0 * DH, (h0 + HPC) * DH)
        m = {"xt": xT}
        bq = np.empty((3, CD), np.float32)
        for row, (name, off) in enumerate(
                (("wq", 0), ("wk", D), ("wv", 2 * D))):
            m[name] = np.ascontiguousarray(w_qkv[:, off + cols])
            bq[row] = b_qkv[off + cols]
        m["bqkv"] = bq
        m["wo"] = np.ascontiguousarray(w_out[cols, :])
        in_maps.append(m)
    return in_maps


def kernel(x, w_qkv, b_qkv, w_out, b_out, _trace=False):
    from concourse.bass_utils import run_bass_kernel_spmd

    if "nc" not in _CACHE:
        _CACHE["nc"] = _build()
    nc = _CACHE["nc"]

    in_maps = _prep_inputs(x, w_qkv, b_qkv, w_out, b_out)
    res = run_bass_kernel_spmd(nc, in_maps, core_ids=list(range(NCORES)),
                               trace=_trace)
    _CACHE["last_result"] = res
    acc = res.results[0]["outt"].astype(np.float32)
    for c in range(1, NCORES):
        acc = acc + res.results[c]["outt"]
    out = acc.T + np.asarray(b_out, np.float32)[None, :]
    return np.ascontiguousarray(out).reshape(1, T, D)



# revision 18
# speedup vs baseline: 1.2546x; 1.2546x over previous
"""Causal self-attention (B=1, T=4096, D=1024, H=16) on 8 TRN2 NeuronCores.

Sharding: tensor-parallel over heads - 2 heads per core. Each core computes
Q^T/K^T/V for its 2 heads from the full x, runs causal attention fully
on-chip, applies its slice of the output projection, and writes a partial
[D, T] (transposed) output. The host sums the 8 partials (the all-reduce of
the out projection) and transposes back.

v2 design (bf16 everywhere):
  - all matmul operands bf16: FWL fast weight loads kick in (compiler-auto
    for non-fp32 128-col weights), DVE gets 2-4x rates, DMA bytes halve.
  - qTs stored block-diagonal [128, 2, T] (h0 rows 0:64 cols block 0,
    h1 rows 64:128 block 1, zeros elsewhere) so one M=1024 matmul with
    lhsT = kT[:, j-block] computes BOTH heads' S^T tile. Output goes to a
    bf16 PSUM tile [128, 2, 512] = 2KB = one bank (single-shot, no
    accumulation, so bf16 PSUM is safe).
  - exp batched: one ACT instr per 2-kv-tile group (N=2048/partition)
    to amortize the 352-cycle ACT overhead.
  - causal mask: only the 128-col window at the diagonal needs masking
    (p <= c triangle, identical for every diagonal tile).
  - rowsum via ones-columns in v: v tile per head is [64 v | 64 ones], so
    the y^T matmul yields rows 0:64 = y^T, rows 64:128 = rowsum replicated
    64x. Normalize = reciprocal_approx_fast [64,512] + one multiply. (The
    old path burned 53us in 1-partition RECIPROCAL + PE broadcasts.)
  - v natural layout via DMA xbar transpose (dma_start_transpose), not PE.
  - out projection DMA'd straight from PSUM (f32), no cast op.
"""

import numpy as np

T = 4096
D = 1024
H = 16
DH = 64
NCORES = 8
HPC = H // NCORES          # heads per core = 2
CD = HPC * DH              # per-core hidden slice = 128
QT = 512                   # query tile
KT = 128                   # kv tile (partition dim of S^T)
NQ = T // QT               # 8 q tiles
TS = 1024                  # projection t-slice
NTS = T // TS              # 4 slices
NKC = D // 128             # 8 contraction chunks of d_model

_CACHE = {}
_DEBUG = False


def _build():
    import concourse.bass as bass
    import concourse.tile as tile
    from concourse import bacc, mybir

    F32 = mybir.dt.float32
    BF16 = mybir.dt.bfloat16
    AF = mybir.ActivationFunctionType

    nc = bacc.Bacc("TRN2", target_bir_lowering=False, debug=False,
                   num_devices=NCORES)

    xT_d = nc.dram_tensor("xt", [D, T], BF16, kind="ExternalInput").ap()
    wq_d = nc.dram_tensor("wq", [D, CD], BF16, kind="ExternalInput").ap()
    wk_d = nc.dram_tensor("wk", [D, CD], BF16, kind="ExternalInput").ap()
    wv_d = nc.dram_tensor("wv", [D, CD], BF16, kind="ExternalInput").ap()
    bq_d = nc.dram_tensor("bqkv", [3, CD], F32, kind="ExternalInput").ap()
    wo_d = nc.dram_tensor("wo", [CD, D], BF16, kind="ExternalInput").ap()
    outT_d = nc.dram_tensor("outt", [D, T], BF16, kind="ExternalOutput").ap()
    if _DEBUG:
        dbg_q = nc.dram_tensor("dbg_q", [128, HPC, T], BF16,
                               kind="ExternalOutput").ap()
        dbg_k = nc.dram_tensor("dbg_k", [128, T], BF16,
                               kind="ExternalOutput").ap()
        dbg_v = nc.dram_tensor("dbg_v", [128, T // KT, HPC, 128], BF16,
                               kind="ExternalOutput").ap()
        dbg_y = nc.dram_tensor("dbg_y", [128, T], BF16,
                               kind="ExternalOutput").ap()
        dbg_mask = nc.dram_tensor("dbg_mask", [128, HPC, KT], BF16,
                                  kind="ExternalOutput").ap()
        dbg_pt0 = nc.dram_tensor("dbg_pt0", [128, HPC, QT], BF16,
                                 kind="ExternalOutput").ap()
        dbg_pt1 = nc.dram_tensor("dbg_pt1", [128, HPC, QT], BF16,
                                 kind="ExternalOutput").ap()
        dbg_rs = nc.dram_tensor("dbg_rs", [64, QT], F32,
                                kind="ExternalOutput").ap()

    with (
        tile.TileContext(nc) as tc,
        tc.tile_pool(name="persist", bufs=1) as persist,
        tc.tile_pool(name="xt", bufs=2) as xtp,
        tc.tile_pool(name="vtq", bufs=2) as vtqp,
        tc.tile_pool(name="pt", bufs=6) as ptp,
        tc.tile_pool(name="rs", bufs=2) as rsp,
        tc.tile_pool(name="ot", bufs=3) as otp,
        tc.tile_pool(name="vtt", bufs=2) as vttp,
        tc.tile_pool(name="ps_big", bufs=3, space="PSUM") as psb,
        tc.tile_pool(name="ps_y", bufs=2, space="PSUM") as psy,
    ):
        # earliest critical path: slice-0 x DMA before all constant DMAs
        _xt0 = xtp.tile([128, NKC, TS], BF16, tag="xt", name="xt0")
        for _k in range(NKC):
            nc.sync.dma_start(out=_xt0[:, _k, :],
                              in_=xT_d[_k * 128:(_k + 1) * 128, 0:TS])

        # ---------------- constants & persistent tiles ----------------
        # causal window mask (keep kv_p <= q_c within the 128-col window;
        # identical for every diagonal tile), replicated for both heads
        mask_f = persist.tile([128, 2, KT], F32)
        nc.vector.memset(mask_f, 1.0)
        for h in range(HPC):
            nc.gpsimd.affine_select(
                out=mask_f[:, h, :], in_=mask_f[:, h, :],
                compare_op=mybir.AluOpType.is_ge, fill=0.0,
                base=0, pattern=[[1, KT]], channel_multiplier=-1,
            )
        mask2 = persist.tile([128, 2, KT], BF16)
        nc.vector.tensor_copy(out=mask2, in_=mask_f)

        wo_sb = persist.tile([128, NKC, 128], BF16)
        nc.sync.dma_start(out=wo_sb, in_=wo_d.rearrange("p (a m) -> p a m", a=NKC))

        # projection weights: [128, chunk, CD] (+ bias rows separately)
        wq_sb = persist.tile([128, NKC, CD], BF16)
        wk_sb = persist.tile([128, NKC, CD], BF16)
        wv_sb = persist.tile([128, NKC, CD], BF16)
        nc.sync.dma_start(out=wq_sb, in_=wq_d.rearrange("(a p) m -> p a m", p=128))
        nc.sync.dma_start(out=wk_sb, in_=wk_d.rearrange("(a p) m -> p a m", p=128))
        nc.sync.dma_start(out=wv_sb, in_=wv_d.rearrange("(a p) m -> p a m", p=128))
        bq_sb = persist.tile([128, 3], F32)
        nc.sync.dma_start(out=bq_sb, in_=bq_d.rearrange("a p -> p a"))

        # persistent activations
        # q block-diagonal: [:, 0, :] rows 0:64 = q_h0 (rest zero),
        #                   [:, 1, :] rows 64:128 = q_h1 (rest zero)
        qTs = persist.tile([128, HPC, T], BF16)
        nc.vector.memset(qTs, 0.0)
        kTs = persist.tile([128, T], BF16)
        yTs = persist.tile([128, T], BF16)
        # v natural per kv tile and head: [64 v | 64 ones]
        v_sb = persist.tile([128, T // KT, HPC, 128], BF16)
        nc.vector.memset(v_sb[:, :, :, DH:], 1.0)

        def xt_dma(ts):
            sl = slice(ts * TS, (ts + 1) * TS)
            xt_e = xtp.tile([128, NKC, TS], BF16, tag="xt", name=f"xt{ts}")
            for k in range(NKC):
                nc.sync.dma_start(
                    out=xt_e[:, k, :],
                    in_=xT_d[k * 128:(k + 1) * 128, sl])
            return xt_e

        def proj_chunks(ts, pre=None):
            """Emit-later closures for projection slice ts (TS=1024 wide,
            matmuls per 512 half): DMA first, then q/k/v matmuls, then v
            transposes via DMA xbar."""
            state = {"xt": pre}

            def dma_chunk():
                state["xt"] = xt_dma(ts)

            def mm_half(which, bidx, w_sb, half):
                sl = slice(ts * TS + half * QT, ts * TS + (half + 1) * QT)

                def emit():
                    xt_e = state["xt"]
                    ps = psb.tile([128, QT], F32, tag="big",
                                  name=f"p{which}{ts}h{half}")
                    for k in range(NKC):
                        nc.tensor.matmul(
                            ps, w_sb[:, k, :],
                            xt_e[:, k, half * QT:(half + 1) * QT],
                            start=(k == 0), stop=(k == NKC - 1))
                    if which == "q":
                        # split write into the block-diagonal layout
                        for h in range(HPC):
                            hs = slice(h * DH, (h + 1) * DH)
                            nc.vector.tensor_scalar_add(
                                out=qTs[hs, h, sl], in0=ps[hs, :],
                                scalar1=bq_sb[hs, bidx:bidx + 1])
                    elif which == "k":
                        nc.vector.tensor_scalar_add(
                            out=kTs[:, sl], in0=ps,
                            scalar1=bq_sb[:, bidx:bidx + 1])
                    else:
                        vt_q = vtqp.tile([128, QT], BF16, tag="vtq",
                                         name=f"vtq{ts}h{half}")
                        nc.vector.tensor_scalar_add(
                            out=vt_q, in0=ps,
                            scalar1=bq_sb[:, bidx:bidx + 1])
                        state[f"vtq{half}"] = vt_q
                return emit

            def tr_half(half):
                def emit():
                    vt_q = state[f"vtq{half}"]
                    nb = QT // KT
                    vt_t = vttp.tile([128, nb, KT], BF16, tag="vtt",
                                     name=f"vtt{ts}h{half}")
                    for jj in range(nb):
                        nc.sync.dma_start_transpose(
                            out=vt_t[:, jj, :],
                            in_=vt_q[:, jj * KT:(jj + 1) * KT])
                    j0 = ts * (TS // KT) + half * nb
                    nc.vector.tensor_copy(
                        out=v_sb[:, j0:j0 + nb, :, 0:DH],
                        in_=vt_t.rearrange("p a (h d) -> p a h d", h=HPC))
                return emit

            head = [] if pre is not None else [dma_chunk]
            return head + [
                mm_half("q", 0, wq_sb, 0), mm_half("q", 0, wq_sb, 1),
                mm_half("k", 1, wk_sb, 0), mm_half("k", 1, wk_sb, 1),
                mm_half("v", 2, wv_sb, 0), tr_half(0),
                mm_half("v", 2, wv_sb, 1), tr_half(1)]

        def attention(i, filler=()):
            filler = list(filler)
            qsl = slice(i * QT, (i + 1) * QT)
            nj = (i + 1) * (QT // KT)           # kv tiles for this q tile
            ngroups = nj // 2
            stride = max(1, ngroups // max(1, len(filler)))
            ys = [psy.tile([128, QT], F32, tag="y", name=f"y{i}h{h}")
                  for h in range(HPC)]
            for g in range(ngroups):
                for jj in range(2):
                    j = 2 * g + jj
                    jr = j - (QT // KT) * i
                    jsl = slice(j * KT, (j + 1) * KT)
                    f0 = max(0, KT * jr)
                    # S^T per head into one f32 PSUM tile [128, 2, 512];
                    # lhsT (full 128-row kT slice) is shared between the
                    # two matmuls - the block-diagonal qTs zeros mask the
                    # other head's rows out of the contraction.
                    sg = psb.tile([128, 2, QT], F32, tag="big",
                                  name=f"sg{i}j{j}")
                    for h in range(HPC):
                        nc.tensor.matmul(
                            sg[:, h, f0:], kTs[:, jsl],
                            qTs[:, h, i * QT + f0:(i + 1) * QT],
                            start=True, stop=True)
                    # one exp per kv tile (lead cols of the h1 half of a
                    # diagonal tile are stale garbage - never read below)
                    pt = ptp.tile([128, 2, QT], BF16, tag="pt",
                                  name=f"pt{i}j{j}")
                    sg_f = sg.rearrange("p a m -> p (a m)")
                    pt_f = pt.rearrange("p a m -> p (a m)")
                    nc.scalar.activation(out=pt_f[:, f0:], in_=sg_f[:, f0:],
                                         func=AF.Exp, scale=0.125)
                    if jr >= 0:   # diagonal tile: apply causal window mask
                        nc.vector.tensor_mul(
                            out=pt[:, :, f0:f0 + KT],
                            in0=pt[:, :, f0:f0 + KT],
                            in1=mask2)
                    if _DEBUG and i == 0 and j == 0:
                        nc.sync.dma_start(out=dbg_pt0, in_=pt)
                    if _DEBUG and i == 1 and j == 0:
                        nc.sync.dma_start(out=dbg_pt1, in_=pt)
                    for h in range(HPC):
                        nc.tensor.matmul(
                            ys[h][:, f0:], v_sb[:, j, h, :],
                            pt[:, h, f0:], start=(j == 0),
                            stop=(j == nj - 1))
                if filler and (g % stride == stride - 1 or g == ngroups - 1):
                    filler.pop(0)()
            while filler:
                filler.pop(0)()
            # normalize: y[0:64] * recip(rowsum replicated in y[64:128])
            for h in range(HPC):
                rs = rsp.tile([64, QT], F32, tag="rs", name=f"rs{i}h{h}")
                with nc.allow_low_precision(reason="bf16-grade kernel"):
                    nc.vector.reciprocal(out=rs, in_=ys[h][64:128, :])
                    nc.vector.tensor_mul(
                        out=yTs[h * DH:(h + 1) * DH, qsl],
                        in0=ys[h][0:64, :], in1=rs)
                if _DEBUG and i == 0 and h == 0:
                    nc.sync.dma_start(out=dbg_rs, in_=rs)
            # out projection chunks for this t slice (emitted as filler)
            outT_r = outT_d.rearrange("(a p) t -> p a t", p=128)

            def po_chunk(dp):
                def emit():
                    po = psb.tile([128, 2, QT], F32, tag="big",
                                  name=f"po{i}d{dp}")
                    for d2 in range(2):
                        d = 2 * dp + d2
                        nc.tensor.matmul(po[:, d2, :], wo_sb[:, d, :],
                                         yTs[:, qsl], start=True, stop=True)
                    ot = otp.tile([128, 2, QT], BF16, tag="ot",
                                  name=f"ot{i}d{dp}")
                    # split PSUM->SBUF casts between the two engines
                    with nc.allow_low_precision(reason="bf16 partials"):
                        if dp % 2 == 0:
                            nc.scalar.copy(out=ot, in_=po)
                        else:
                            nc.vector.tensor_copy(out=ot, in_=po)
                    nc.sync.dma_start(
                        out=outT_r[:, 2 * dp:2 * dp + 2, qsl], in_=ot)
                return emit

            return [po_chunk(dp) for dp in range(NKC // 2)]

        # proj slice 0 up front; proj slice s is interleaved into
        # attention(2s-2)/(2s-1); outproj(i-1) into attention(i)
        for ch in proj_chunks(0, pre=_xt0):
            ch()
        carry = []
        pending_proj = {}
        for i in range(NQ):
            s = i // 2 + 1
            if s < NTS:
                if i % 2 == 0:
                    chunks = proj_chunks(s)
                    half = len(chunks) // 2 + 1
                    nxt = chunks[:half]
                    pending_proj[s] = chunks[half:]
                else:
                    nxt = pending_proj.pop(s)
            else:
                nxt = []
            filler = nxt[:1]
            rest = nxt[1:]
            while carry or rest:
                if carry:
                    filler.append(carry.pop(0))
                if rest:
                    filler.append(rest.pop(0))
            carry = attention(i, filler)
        for ch in carry:
            ch()
        if _DEBUG:
            nc.sync.dma_start(out=dbg_q, in_=qTs)
            nc.sync.dma_start(out=dbg_k, in_=kTs)
            nc.sync.dma_start(out=dbg_v, in_=v_sb)
            nc.sync.dma_start(out=dbg_y, in_=yTs)
            nc.sync.dma_start(out=dbg_mask, in_=mask2)

    nc.compile()
    return nc


def _prep_inputs(x, w_qkv, b_qkv, w_out, b_out):
    import ml_dtypes

    BF = ml_dtypes.bfloat16
    x = np.asarray(x, dtype=np.float32).reshape(T, D)
    w_qkv = np.asarray(w_qkv, dtype=np.float32)
    b_qkv = np.asarray(b_qkv, dtype=np.float32)
    w_out = np.asarray(w_out, dtype=np.float32)

    xT = np.ascontiguousarray(x.T).astype(BF)

    in_maps = []
    for c in range(NCORES):
        h0 = HPC * c
        cols = np.arange(h0 * DH, (h0 + HPC) * DH)
        m = {"xt": xT}
        bq = np.empty((3, CD), np.float32)
        for row, (name, off) in enumerate(
                (("wq", 0), ("wk", D), ("wv", 2 * D))):
            m[name] = np.ascontiguousarray(w_qkv[:, off + cols]).astype(BF)
            bq[row] = b_qkv[off + cols]
        m["bqkv"] = bq
        m["wo"] = np.ascontiguousarray(w_out[cols, :]).astype(BF)
        in_maps.append(m)
    return in_maps


def kernel(x, w_qkv, b_qkv, w_out, b_out, _trace=False):
    from concourse.bass_utils import run_bass_kernel_spmd

    if "nc" not in _CACHE:
        _CACHE["nc"] = _build()
    nc = _CACHE["nc"]

    in_maps = _prep_inputs(x, w_qkv, b_qkv, w_out, b_out)
    res = run_bass_kernel_spmd(nc, in_maps, core_ids=list(range(NCORES)),
                               trace=_trace)
    _CACHE["last_result"] = res
    acc = res.results[0]["outt"].astype(np.float32)
    for c in range(1, NCORES):
        acc = acc + res.results[c]["outt"]
    out = acc.T + np.asarray(b_out, np.float32)[None, :]
    return np.ascontiguousarray(out).reshape(1, T, D)
